# revision 20
# baseline (speedup 1.0000x reference)
"""BatchHardTripletLoss on 8 Trainium2 NeuronCores.

Strategy (batch/row sharding): core c owns anchor rows [512c, 512c+512).
All tensors are rolled by 512c rows on the host so local row i == global
row 512c+i and the self-match diagonal is at a static column block.

Score space: s_j = a.y_j - 0.5(||y_j||^2 - 128), so the hardest-negative
distance is d2_min = a2 + 128 - 2*max_j s_j.

Candidate-pair max trick (the drain is the bottleneck — PSUM is readable
only by VectorE/ScalarE at 1 elem/cycle/lane): the host pairs adjacent
candidates and ships ysum=(y+y')/2, ydif=(y-y')/2 (fp8) plus
error-feedback-quantized square-rows qsum/qdif.  For each pair column,
PE DoubleRow matmuls produce psum_sum=(s+s')/2 and psum_dif=(s-s')/2;
ScalarE computes |psum_dif| (ACT Abs -> fp16 SBUF, its only big job);
PE adds it back with an identity matmul so PSUM holds max(s,s') —
HALF the columns ever drained; VectorE direct-reduces those.  The
self-match mask adds -224 to the sum col and -+224 (slot-parity sign) to
the dif col; since -448 dominates, max(s-448, s') == s' exactly through
the abs.

Row stats a2/dpsq come from fp32 slices via per-block column-sum
matmuls (exact).  Tail: Sqrt on ScalarE (one act table), softplus(x) as
max(x,0) + Pade33(ln(1+e^-|x|)) so only Exp is ever loaded afterwards.
Each core emits the sum of its 512 row losses; the host sums 8 partials
and divides by 4096.
"""

import os
import sys

if "/opt/trn_rl_repo" not in sys.path:
    sys.path.insert(0, "/opt/trn_rl_repo")

from contextlib import ExitStack

import numpy as np
import ml_dtypes

import concourse.bass as bass
import concourse.tile as tile
from concourse import bacc, bass_utils, mybir

F32 = mybir.dt.float32
F16 = mybir.dt.float16
F8 = mybir.dt.float8e4
BF16 = mybir.dt.bfloat16
AF = mybir.ActivationFunctionType
ALU = mybir.AluOpType
DR = mybir.MatmulPerfMode.DoubleRow
# e4m3fn shares encodings with e4m3 for |v| <= 240 (all values used here)
NPF8 = ml_dtypes.float8_e4m3fn

B, D, NCORES = 4096, 128, 8
RB = B // NCORES        # 512 rows per core
MT = RB // 128          # 4 m-blocks per core
NP = B // 2             # 2048 candidate pairs per tensor
EPS = 1e-12
NEG = -3.0e38

_CACHE: dict = {}


def _build():
    nc = bacc.Bacc("TRN2", target_bir_lowering=False, debug=False)

    lhs_d = nc.dram_tensor("lhspack", [128, MT, 2, 128], F8,
                           kind="ExternalInput").ap()
    eye_d = nc.dram_tensor("eyepack", [128, 2, 128], F8,
                           kind="ExternalInput").ap()
    eya_d = nc.dram_tensor("eyealt", [128, 2, 128], F8,
                           kind="ExternalInput").ap()
    eyi_d = nc.dram_tensor("eyeid", [128, 128], F16,
                           kind="ExternalInput").ap()
    ibf_d = nc.dram_tensor("ibufpack", [128, 2, 1024], F8,
                           kind="ExternalInput").ap()
    asl_d = nc.dram_tensor("aslice", [128, RB], F32, kind="ExternalInput").ap()
    psl_d = nc.dram_tensor("pslice", [128, RB], F32, kind="ExternalInput").ap()
    # per tensor: ktile0 = [ysum | ydif] cols, ktile1 = [qsum | qdif]
    ypk_d = [nc.dram_tensor(f"ypk{y}", [128, 2, B], F8,
                            kind="ExternalInput").ap() for y in range(3)]
    out_d = nc.dram_tensor("out", [1, 1], F32, kind="ExternalOutput").ap()

    with tile.TileContext(nc) as tc:
        with ExitStack() as ctx:
            _emit(ctx, tc, nc, lhs_d, eye_d, eya_d, eyi_d, ibf_d,
                  asl_d, psl_d, ypk_d, out_d)
    nc.compile()
    return nc


def _emit(ctx, tc, nc, lhs_d, eye_d, eya_d, eyi_d, ibf_d, asl_d, psl_d,
          ypk_d, out_d):
    const = ctx.enter_context(tc.tile_pool(name="const", bufs=1))
    inp = ctx.enter_context(tc.tile_pool(name="inp", bufs=1))
    adp = ctx.enter_context(tc.tile_pool(name="adp", bufs=3))
    fin = ctx.enter_context(tc.tile_pool(name="fin", bufs=1))
    scr = ctx.enter_context(tc.tile_pool(name="scr", bufs=2))
    spool = ctx.enter_context(tc.tile_pool(name="spool", bufs=2, space="PSUM"))
    dpool = ctx.enter_context(tc.tile_pool(name="dpool", bufs=2, space="PSUM"))

    lhsp = inp.tile([128, MT, 2, 128], F8, tag="lhsp")
    eyep = inp.tile([128, 2, 128], F8, tag="eyep")
    eyea = inp.tile([128, 2, 128], F8, tag="eyea")
    eyei = inp.tile([128, 128], F16, tag="eyei")
    ibufp = inp.tile([128, 2, 1024], F8, tag="ibufp")
    asl = inp.tile([128, RB], F32, tag="asl")
    psl = inp.tile([128, RB], F32, tag="psl")
    ypk = [inp.tile([128, 2, B], F8, tag=f"ypk{y}", name=f"ypk{y}")
           for y in range(3)]

    ones_col = const.tile([128, 1], F32, tag="ones_col")
    nc.vector.memset(ones_col[:], 1.0)

    # ---- input DMAs: tiny weights + stats slices first (they fill the
    #      DMA ramp while nothing can run), then ypk in use order ----
    nc.sync.dma_start(lhsp[:], lhs_d)
    nc.sync.dma_start(eyep[:], eye_d)
    nc.sync.dma_start(eyea[:], eya_d)
    nc.sync.dma_start(eyei[:], eyi_d)
    nc.sync.dma_start(ibufp[:], ibf_d)
    nc.sync.dma_start(ypk[0][:, :, 2048:3072], ypk_d[0][:, :, 2048:3072])
    nc.sync.dma_start(ypk[0][:, :, 0:1024], ypk_d[0][:, :, 0:1024])
    nc.sync.dma_start(ypk[0][:, :, 3072:4096], ypk_d[0][:, :, 3072:4096])
    nc.sync.dma_start(ypk[0][:, :, 1024:2048], ypk_d[0][:, :, 1024:2048])
    nc.sync.dma_start(ypk[1][:], ypk_d[1])
    nc.sync.dma_start(asl[:], asl_d)
    nc.sync.dma_start(psl[:], psl_d)
    nc.sync.dma_start(ypk[2][:], ypk_d[2])

    scol = fin.tile([128, 2 * MT], F32, tag="scol")

    def emit_stats():
        # row stats: a2 / dpsq by per-block column-sum matmuls (exact)
        asq = scr.tile([128, RB], F32, tag="asq")
        nc.scalar.activation(asq[:], asl[:], AF.Square)
        dif = scr.tile([128, RB], F32, tag="dif")
        nc.vector.tensor_tensor(out=dif[:], in0=asl[:], in1=psl[:],
                                op=ALU.subtract)
        difsq = scr.tile([128, RB], F32, tag="difsq")
        nc.scalar.activation(difsq[:], dif[:], AF.Square)
        sp = spool.tile([128, 1024], F32, tag="sum", name="spstats")
        for m in range(MT):
            nc.tensor.matmul(sp[:, m:m + 1], asq[:, 128 * m:128 * (m + 1)],
                             ones_col[:], start=(m == 0), stop=False)
        for m in range(MT):
            nc.tensor.matmul(sp[:, MT + m:MT + m + 1],
                             difsq[:, 128 * m:128 * (m + 1)], ones_col[:],
                             start=False, stop=(m == MT - 1))
        nc.scalar.activation(scol[:], sp[:, 0:2 * MT], AF.Copy)
        nc.vector.tensor_scalar_max(out=scol[:, MT:2 * MT],
                                    in0=scol[:, MT:2 * MT], scalar1=EPS)

    # ---- working tiles for the reduction/tail ----
    vcol = fin.tile([128, 8 * MT], F32, tag="vcol")
    nc.vector.memset(vcol[:], NEG)
    maxv = fin.tile([128, MT], F32, tag="maxv")
    hnsq = fin.tile([128, MT], F32, tag="hnsq")
    hn = fin.tile([128, MT], F32, tag="hn")
    dp = fin.tile([128, MT], F32, tag="dp")
    xcol = fin.tile([128, MT], F32, tag="xcol")

    sqrt_instrs = []
    slot = {m: 0 for m in range(MT)}
    done = {m: 0 for m in range(MT)}

    def vslot(m):
        s = slot[m]
        slot[m] += 1
        assert s < 8
        return vcol[:, 8 * m + s:8 * m + s + 1]

    def m_tail(m):
        # hardest-neg^2 for block m, then Sqrt (table loaded once at m0)
        nc.vector.tensor_reduce(out=maxv[:, m:m + 1],
                                in_=vcol[:, 8 * m:8 * m + 8],
                                axis=mybir.AxisListType.X, op=ALU.max)
        nc.vector.tensor_scalar(out=hnsq[:, m:m + 1], in0=maxv[:, m:m + 1],
                                scalar1=-2.0, scalar2=128.0, op0=ALU.mult,
                                op1=ALU.add)
        nc.vector.tensor_tensor(out=hnsq[:, m:m + 1], in0=hnsq[:, m:m + 1],
                                in1=scol[:, m:m + 1], op=ALU.add)
        nc.vector.tensor_scalar_max(out=hnsq[:, m:m + 1],
                                    in0=hnsq[:, m:m + 1], scalar1=EPS)
        sqrt_instrs.append(
            nc.scalar.activation(hn[:, m:m + 1], hnsq[:, m:m + 1], AF.Sqrt))
        sqrt_instrs.append(
            nc.scalar.activation(dp[:, m:m + 1],
                                 scol[:, MT + m:MT + m + 1], AF.Sqrt))
        nc.vector.tensor_tensor(out=xcol[:, m:m + 1], in0=dp[:, m:m + 1],
                                in1=hn[:, m:m + 1], op=ALU.subtract)

    # deferred finish of a unit: identity-add |dif| into the sum banks
    # (closing their accumulation groups), then drain with one reduce
    def finish(prev):
        gs, ad, m = prev
        for k in range(2):
            nc.tensor.matmul(gs[:, 512 * k:512 * (k + 1)], eyei[:],
                             ad[:, 512 * k:512 * (k + 1)],
                             start=False, stop=True)
        nc.vector.tensor_reduce(out=vslot(m), in_=gs[:],
                                axis=mybir.AxisListType.X, op=ALU.max)
        done[m] += 1
        if done[m] == 6:
            m_tail(m)

    # ---- main loop: 24 units of [128, 1024] sum + [128, 1024] dif psum;
    #      dif dies at the abs, sum lives until the reduce, so they rotate
    #      in independent 2-deep pools to hide the PE->S->PE->V latency ----
    prev = None
    unit = 0
    for y in range(3):
        for m in range(MT):
            for h in range(2):
                masked = (h == 0 and y < 2)
                gd = dpool.tile([128, 1024], F32, tag="dif",
                                name=f"gd{y}{m}{h}")
                gs = spool.tile([128, 1024], F32, tag="sum",
                                name=f"gs{y}{m}{h}")
                # all four DR matmuls share lhsp[:, m] (one weight load);
                # masks accumulate afterwards with their own weights
                for k in range(2):
                    c = 2048 + 1024 * h + 512 * k
                    nc.tensor.matmul(gd[:, 512 * k:512 * (k + 1)],
                                     lhsp[:, m], ypk[y][:, :, c:c + 512],
                                     start=True,
                                     stop=not (masked and k == 0),
                                     perf_mode=DR)
                for k in range(2):     # left open for finish()
                    c = 1024 * h + 512 * k
                    nc.tensor.matmul(gs[:, 512 * k:512 * (k + 1)], lhsp[:, m],
                                     ypk[y][:, :, c:c + 512],
                                     start=True, stop=False, perf_mode=DR)
                if masked:
                    # -+224 (slot parity) on the diag pair's dif col,
                    # -224 on its sum col
                    nc.tensor.matmul(gd[:, 0:512], eyea[:],
                                     ibufp[:, :, 512 - 64 * m:1024 - 64 * m],
                                     start=False, stop=True, perf_mode=DR)
                    nc.tensor.matmul(gs[:, 0:512], eyep[:],
                                     ibufp[:, :, 512 - 64 * m:1024 - 64 * m],
                                     start=False, stop=False, perf_mode=DR)
                ad = adp.tile([128, 1024], F16, tag="ad", name=f"ad{y}{m}{h}")
                nc.scalar.activation(ad[:], gd[:], AF.Abs)
                if prev is not None:
                    finish(prev)
                prev = (gs, ad, m)
                unit += 1
                if unit == 7:
                    emit_stats()
    finish(prev)

    # ---- softplus tail: loss = max(x,0) + Pade33(ln(1+u)), u = e^-|x| ----
    zneg = fin.tile([128, MT], F32, tag="zneg")
    nc.vector.tensor_scalar(out=zneg[:], in0=xcol[:], scalar1=-1.0,
                            scalar2=None, op0=ALU.mult)
    nc.vector.tensor_tensor(out=zneg[:], in0=xcol[:], in1=zneg[:], op=ALU.min)
    u = fin.tile([128, MT], F32, tag="u")
    i_exp = nc.scalar.activation(u[:], zneg[:], AF.Exp)
    # keep the Exp after every Sqrt so the act table only switches once
    from concourse.bass import _add_dep_helper
    for si in sqrt_instrs:
        _add_dep_helper(i_exp.ins, si.ins, sync=False, reason="act table order")

    u2 = fin.tile([128, MT], F32, tag="u2")
    nc.vector.tensor_tensor(out=u2[:], in0=u[:], in1=u[:], op=ALU.mult)
    t1 = fin.tile([128, MT], F32, tag="t1")
    nc.vector.tensor_scalar(out=t1[:], in0=u[:], scalar1=60.0, scalar2=60.0,
                            op0=ALU.mult, op1=ALU.add)
    t2 = fin.tile([128, MT], F32, tag="t2")
    nc.vector.tensor_scalar(out=t2[:], in0=u2[:], scalar1=11.0, scalar2=None,
                            op0=ALU.mult)
    nc.vector.tensor_tensor(out=t1[:], in0=t1[:], in1=t2[:], op=ALU.add)
    num = fin.tile([128, MT], F32, tag="num")
    nc.vector.tensor_tensor(out=num[:], in0=t1[:], in1=u[:], op=ALU.mult)
    den = fin.tile([128, MT], F32, tag="den")
    nc.vector.tensor_scalar(out=den[:], in0=u[:], scalar1=90.0, scalar2=60.0,
                            op0=ALU.mult, op1=ALU.add)
    nc.vector.tensor_scalar(out=t2[:], in0=u[:], scalar1=3.0, scalar2=36.0,
                            op0=ALU.mult, op1=ALU.add)
    nc.vector.tensor_tensor(out=t2[:], in0=t2[:], in1=u2[:], op=ALU.mult)
    nc.vector.tensor_tensor(out=den[:], in0=den[:], in1=t2[:], op=ALU.add)
    rden = fin.tile([128, MT], F32, tag="rden")
    nc.vector.reciprocal(rden[:], den[:])
    lg = fin.tile([128, MT], F32, tag="lg")
    nc.vector.tensor_tensor(out=lg[:], in0=num[:], in1=rden[:], op=ALU.mult)
    relu = fin.tile([128, MT], F32, tag="relu")
    nc.vector.tensor_scalar_max(out=relu[:], in0=xcol[:], scalar1=0.0)
    nc.vector.tensor_tensor(out=lg[:], in0=lg[:], in1=relu[:], op=ALU.add)
    lsum = fin.tile([128, 1], F32, tag="lsum")
    nc.vector.tensor_reduce(out=lsum[:], in_=lg[:],
                            axis=mybir.AxisListType.X, op=ALU.add)
    ps = spool.tile([128, 1024], F32, tag="sum", name="psfinal")
    nc.tensor.matmul(ps[0:1, 0:1], lsum[:], ones_col[:], start=True, stop=True)
    res = fin.tile([1, 1], F32, tag="res")
    nc.scalar.activation(res[:], ps[0:1, 0:1], AF.Copy)
    nc.sync.dma_start(out_d, res[:])


def _get_nc():
    if "nc" not in _CACHE:
        _CACHE["nc"] = _build()
    return _CACHE["nc"]


def _feedback_quant(x):
    """fp8-quantize rows of x with error feedback along the last axis so
    each row's fp8 sum tracks the fp32 row sum."""
    out = np.empty(x.shape, dtype=NPF8)
    carry = np.zeros(x.shape[0], dtype=np.float32)
    for d in range(x.shape[1]):
        v = x[:, d] + carry
        q = v.astype(NPF8)
        out[:, d] = q
        carry = v - q.astype(np.float32)
    return out


def _host_pack(A, P, N):
    Ys = [A, P, N]
    A8 = A.astype(NPF8)

    eye = (np.eye(128, dtype=np.float32) * -224.0).astype(NPF8)
    eyepack = np.ascontiguousarray(np.stack([eye, eye], axis=1))
    sgn = np.where(np.arange(128) % 2 == 0, -224.0, 224.0).astype(np.float32)
    eya = (np.diag(sgn)).astype(NPF8)
    eyealt = np.ascontiguousarray(np.stack([eya, eya], axis=1))
    eyeid = np.eye(128, dtype=np.float16)
    # row d: indicator of its pair column at 512 + d//2; ktile1 = zeros
    ib = np.zeros((128, 2, 1024), dtype=np.float32)
    ib[np.arange(128), 0, 512 + np.arange(128) // 2] = 1.0
    ibufpack = ib.astype(NPF8)

    in_maps = []
    for c in range(NCORES):
        r = RB * c
        idx = np.r_[r:B, 0:r]
        m = {"eyepack": eyepack, "eyealt": eyealt, "eyeid": eyeid,
             "ibufpack": ibufpack}
        for y in range(3):
            Yr = Ys[y][idx]
            ysum = ((Yr[0::2] + Yr[1::2]) * 0.5).astype(NPF8)
            ydif = ((Yr[0::2] - Yr[1::2]) * 0.5).astype(NPF8)
            ysq = Yr * Yr
            qsum = _feedback_quant((ysq[0::2] + ysq[1::2]) * 0.5 - 1.0)
            qdif = _feedback_quant((ysq[0::2] - ysq[1::2]) * 0.5)
            k0 = np.concatenate([ysum.T, ydif.T], axis=1)   # [128, 4096]
            k1 = np.concatenate([qsum.T, qdif.T], axis=1)
            m[f"ypk{y}"] = np.ascontiguousarray(
                np.stack([k0, k1], axis=1)).astype(NPF8)
        ownT = A8[idx][:RB].T          # [128, 512] fp8
        lhspack = np.empty((128, MT, 2, 128), dtype=NPF8)
        for mm in range(MT):
            lhspack[:, mm, 0, :] = ownT[:, 128 * mm:128 * (mm + 1)]
        lhspack[:, :, 1, :] = np.float32(-0.5)
        m["lhspack"] = lhspack
        m["aslice"] = np.ascontiguousarray(A[idx][:RB].T)
        m["pslice"] = np.ascontiguousarray(P[idx][:RB].T)
        in_maps.append(m)
    return in_maps


def kernel(rep_anchor, rep_pos, rep_neg):
    A = np.ascontiguousarray(rep_anchor, dtype=np.float32)
    P = np.ascontiguousarray(rep_pos, dtype=np.float32)
    N = np.ascontiguousarray(rep_neg, dtype=np.float32)

    nc = _get_nc()
    in_maps = _host_pack(A, P, N)
    res = bass_utils.run_bass_kernel_spmd(nc, in_maps,
                                          core_ids=list(range(NCORES)))
    total = np.float64(0.0)
    for c in range(NCORES):
        total += np.float64(res.results[c]["out"][0, 0])
    return np.float32(total / B)


# revision 22
# speedup vs baseline: 1.1690x; 1.1690x over previous
"""BatchHardTripletLoss on 8 Trainium2 NeuronCores.

Strategy (batch/row sharding): core c owns anchor rows [512c, 512c+512).
All tensors are rolled by 512c rows on the host so local row i == global
row 512c+i and the self-match diagonal is at a static column block.

Score space: s_j = a.y_j - 0.5(||y_j||^2 - 128), so the hardest-negative
distance is d2_min = a2 + 128 - 2*max_j s_j.

Candidate-pair max trick (the drain is the bottleneck — PSUM is readable
only by VectorE/ScalarE at 1 elem/cycle/lane): the host pairs adjacent
candidates and ships ysum=(y+y')/2, ydif=(y-y')/2 (fp8) plus
error-feedback-quantized square-rows qsum/qdif.  For each pair column,
PE DoubleRow matmuls produce psum_sum=(s+s')/2 and psum_dif=(s-s')/2;
ScalarE computes |psum_dif| (ACT Abs -> fp16 SBUF, its only big job);
PE adds it back with an identity matmul so PSUM holds max(s,s') —
HALF the columns ever drained; VectorE direct-reduces those.  The
self-match mask adds -224 to the sum col and -+224 (slot-parity sign) to
the dif col; since -448 dominates, max(s-448, s') == s' exactly through
the abs.

Row stats a2/dpsq come from fp32 slices via per-block column-sum
matmuls (exact).  Tail: Sqrt on ScalarE (one act table), softplus(x) as
max(x,0) + Pade33(ln(1+e^-|x|)) so only Exp is ever loaded afterwards.
Each core emits the sum of its 512 row losses; the host sums 8 partials
and divides by 4096.
"""

import os
import sys

if "/opt/trn_rl_repo" not in sys.path:
    sys.path.insert(0, "/opt/trn_rl_repo")

from contextlib import ExitStack

import numpy as np
import ml_dtypes

import concourse.bass as bass
import concourse.tile as tile
from concourse import bacc, bass_utils, mybir

F32 = mybir.dt.float32
F16 = mybir.dt.float16
F8 = mybir.dt.float8e4
BF16 = mybir.dt.bfloat16
AF = mybir.ActivationFunctionType
ALU = mybir.AluOpType
DR = mybir.MatmulPerfMode.DoubleRow
# e4m3fn shares encodings with e4m3 for |v| <= 240 (all values used here)
NPF8 = ml_dtypes.float8_e4m3fn

B, D, NCORES = 4096, 128, 8
RB = B // NCORES        # 512 rows per core
MT = RB // 128          # 4 m-blocks per core
NP = B // 2             # 2048 candidate pairs per tensor
EPS = 1e-12
NEG = -3.0e38

_CACHE: dict = {}


def _build():
    nc = bacc.Bacc("TRN2", target_bir_lowering=False, debug=False)

    lhs_d = nc.dram_tensor("lhspack", [128, MT, 2, 128], F8,
                           kind="ExternalInput").ap()
    eye_d = nc.dram_tensor("eyepack", [128, 2, 128], F8,
                           kind="ExternalInput").ap()
    eya_d = nc.dram_tensor("eyealt", [128, 2, 128], F8,
                           kind="ExternalInput").ap()
    eyi_d = nc.dram_tensor("eyeid", [128, 128], F16,
                           kind="ExternalInput").ap()
    ibf_d = nc.dram_tensor("ibufpack", [128, 2, 1024], F8,
                           kind="ExternalInput").ap()
    asl_d = nc.dram_tensor("aslice", [128, RB], F32, kind="ExternalInput").ap()
    psl_d = nc.dram_tensor("pslice", [128, RB], F32, kind="ExternalInput").ap()
    # per tensor: ktile0 = [ysum | ydif] cols, ktile1 = [qsum | qdif]
    ypk_d = [nc.dram_tensor(f"ypk{y}", [128, 2, B], F8,
                            kind="ExternalInput").ap() for y in range(3)]
    out_d = nc.dram_tensor("out", [1, 1], F32, kind="ExternalOutput").ap()

    with tile.TileContext(nc) as tc:
        with ExitStack() as ctx:
            _emit(ctx, tc, nc, lhs_d, eye_d, eya_d, eyi_d, ibf_d,
                  asl_d, psl_d, ypk_d, out_d)
    nc.compile()
    return nc


def _emit(ctx, tc, nc, lhs_d, eye_d, eya_d, eyi_d, ibf_d, asl_d, psl_d,
          ypk_d, out_d):
    const = ctx.enter_context(tc.tile_pool(name="const", bufs=1))
    inp = ctx.enter_context(tc.tile_pool(name="inp", bufs=1))
    adp = ctx.enter_context(tc.tile_pool(name="adp", bufs=3))
    fin = ctx.enter_context(tc.tile_pool(name="fin", bufs=1))
    scr = ctx.enter_context(tc.tile_pool(name="scr", bufs=2))
    spool = ctx.enter_context(tc.tile_pool(name="spool", bufs=2, space="PSUM"))
    dpool = ctx.enter_context(tc.tile_pool(name="dpool", bufs=2, space="PSUM"))

    lhsp = inp.tile([128, MT, 2, 128], F8, tag="lhsp")
    eyep = inp.tile([128, 2, 128], F8, tag="eyep")
    eyea = inp.tile([128, 2, 128], F8, tag="eyea")
    eyei = inp.tile([128, 128], F16, tag="eyei")
    ibufp = inp.tile([128, 2, 1024], F8, tag="ibufp")
    asl = inp.tile([128, RB], F32, tag="asl")
    psl = inp.tile([128, RB], F32, tag="psl")
    ypk = [inp.tile([128, 2, B], F8, tag=f"ypk{y}", name=f"ypk{y}")
           for y in range(3)]

    ones_col = const.tile([128, 1], F32, tag="ones_col")
    nc.vector.memset(ones_col[:], 1.0)

    # ---- input DMAs: tiny weights + stats slices first (they fill the
    #      DMA ramp while nothing can run), then ypk in use order ----
    nc.sync.dma_start(lhsp[:], lhs_d)
    nc.sync.dma_start(eyep[:], eye_d)
    nc.sync.dma_start(eyea[:], eya_d)
    nc.sync.dma_start(eyei[:], eyi_d)
    nc.sync.dma_start(ibufp[:], ibf_d)
    nc.sync.dma_start(ypk[0][:, :, 2048:3072], ypk_d[0][:, :, 2048:3072])
    nc.sync.dma_start(ypk[0][:, :, 0:1024], ypk_d[0][:, :, 0:1024])
    nc.sync.dma_start(ypk[0][:, :, 3072:4096], ypk_d[0][:, :, 3072:4096])
    nc.sync.dma_start(ypk[0][:, :, 1024:2048], ypk_d[0][:, :, 1024:2048])
    nc.sync.dma_start(ypk[1][:], ypk_d[1])
    nc.sync.dma_start(asl[:], asl_d)
    nc.sync.dma_start(psl[:], psl_d)
    nc.sync.dma_start(ypk[2][:], ypk_d[2])

    scol = fin.tile([128, 2 * MT], F32, tag="scol")

    def emit_stats():
        # row stats: a2 / dpsq by per-block column-sum matmuls (exact)
        asq = scr.tile([128, RB], F32, tag="asq")
        nc.scalar.activation(asq[:], asl[:], AF.Square)
        dif = scr.tile([128, RB], F32, tag="dif")
        nc.vector.tensor_tensor(out=dif[:], in0=asl[:], in1=psl[:],
                                op=ALU.subtract)
        difsq = scr.tile([128, RB], F32, tag="difsq")
        nc.scalar.activation(difsq[:], dif[:], AF.Square)
        sp = dpool.tile([128, 1024], F32, tag="dif", name="spstats")
        for m in range(MT):
            nc.tensor.matmul(sp[:, m:m + 1], asq[:, 128 * m:128 * (m + 1)],
                             ones_col[:], start=(m == 0), stop=False)
        for m in range(MT):
            nc.tensor.matmul(sp[:, MT + m:MT + m + 1],
                             difsq[:, 128 * m:128 * (m + 1)], ones_col[:],
                             start=False, stop=(m == MT - 1))
        nc.scalar.activation(scol[:], sp[:, 0:2 * MT], AF.Copy)
        nc.vector.tensor_scalar_max(out=scol[:, MT:2 * MT],
                                    in0=scol[:, MT:2 * MT], scalar1=EPS)

    # ---- working tiles for the reduction/tail ----
    vcol = fin.tile([128, 8 * MT], F32, tag="vcol")
    nc.vector.memset(vcol[:], NEG)
    maxv = fin.tile([128, MT], F32, tag="maxv")
    hnsq = fin.tile([128, MT], F32, tag="hnsq")
    hn = fin.tile([128, MT], F32, tag="hn")
    dp = fin.tile([128, MT], F32, tag="dp")
    xcol = fin.tile([128, MT], F32, tag="xcol")

    sqrt_instrs = []
    slot = {m: 0 for m in range(MT)}
    done = {m: 0 for m in range(MT)}

    def vslot(m):
        s = slot[m]
        slot[m] += 1
        assert s < 8
        return vcol[:, 8 * m + s:8 * m + s + 1]

    def m_tail(m):
        # hardest-neg^2 for block m, then Sqrt (table loaded once at m0)
        nc.vector.tensor_reduce(out=maxv[:, m:m + 1],
                                in_=vcol[:, 8 * m:8 * m + 8],
                                axis=mybir.AxisListType.X, op=ALU.max)
        nc.vector.tensor_scalar(out=hnsq[:, m:m + 1], in0=maxv[:, m:m + 1],
                                scalar1=-2.0, scalar2=128.0, op0=ALU.mult,
                                op1=ALU.add)
        nc.vector.tensor_tensor(out=hnsq[:, m:m + 1], in0=hnsq[:, m:m + 1],
                                in1=scol[:, m:m + 1], op=ALU.add)
        nc.vector.tensor_scalar_max(out=hnsq[:, m:m + 1],
                                    in0=hnsq[:, m:m + 1], scalar1=EPS)
        sqrt_instrs.append(
            nc.scalar.activation(hn[:, m:m + 1], hnsq[:, m:m + 1], AF.Sqrt))
        sqrt_instrs.append(
            nc.scalar.activation(dp[:, m:m + 1],
                                 scol[:, MT + m:MT + m + 1], AF.Sqrt))
        nc.vector.tensor_tensor(out=xcol[:, m:m + 1], in0=dp[:, m:m + 1],
                                in1=hn[:, m:m + 1], op=ALU.subtract)

    # deferred finish of a unit: identity-add |dif| into the sum banks
    # (closing their accumulation groups), then drain with one reduce
    def finish(prev):
        gs, ad, m = prev
        for k in range(2):
            nc.tensor.matmul(gs[:, 512 * k:512 * (k + 1)], eyei[:],
                             ad[:, 512 * k:512 * (k + 1)],
                             start=False, stop=True)
        nc.vector.tensor_reduce(out=vslot(m), in_=gs[:],
                                axis=mybir.AxisListType.X, op=ALU.max)
        done[m] += 1
        if done[m] == 6:
            m_tail(m)

    # ---- main loop: 24 units of [128, 1024] sum + [128, 1024] dif psum;
    #      dif dies at the abs, sum lives until the reduce, so they rotate
    #      in independent 2-deep pools to hide the PE->S->PE->V latency ----
    prev = None
    unit = 0
    for y in range(3):
        for m in range(MT):
            for h in range(2):
                masked = (h == 0 and y < 2)
                gd = dpool.tile([128, 1024], F32, tag="dif",
                                name=f"gd{y}{m}{h}")
                gs = spool.tile([128, 1024], F32, tag="sum",
                                name=f"gs{y}{m}{h}")
                # all four DR matmuls share lhsp[:, m] (one weight load);
                # masks accumulate afterwards with their own weights
                for k in range(2):
                    c = 2048 + 1024 * h + 512 * k
                    nc.tensor.matmul(gd[:, 512 * k:512 * (k + 1)],
                                     lhsp[:, m], ypk[y][:, :, c:c + 512],
                                     start=True,
                                     stop=not (masked and k == 0),
                                     perf_mode=DR)
                for k in range(2):     # left open for finish()
                    c = 1024 * h + 512 * k
                    nc.tensor.matmul(gs[:, 512 * k:512 * (k + 1)], lhsp[:, m],
                                     ypk[y][:, :, c:c + 512],
                                     start=True, stop=False, perf_mode=DR)
                if masked:
                    # -+224 (slot parity) on the diag pair's dif col,
                    # -224 on its sum col
                    nc.tensor.matmul(gd[:, 0:512], eyea[:],
                                     ibufp[:, :, 512 - 64 * m:1024 - 64 * m],
                                     start=False, stop=True, perf_mode=DR)
                    nc.tensor.matmul(gs[:, 0:512], eyep[:],
                                     ibufp[:, :, 512 - 64 * m:1024 - 64 * m],
                                     start=False, stop=False, perf_mode=DR)
                ad = adp.tile([128, 1024], F16, tag="ad", name=f"ad{y}{m}{h}")
                nc.scalar.activation(ad[:], gd[:], AF.Abs)
                if prev is not None:
                    finish(prev)
                prev = (gs, ad, m)
                unit += 1
                if unit == 7:
                    emit_stats()
    finish(prev)

    # ---- softplus tail: loss = max(x,0) + Pade33(ln(1+u)), u = e^-|x| ----
    zneg = fin.tile([128, MT], F32, tag="zneg")
    nc.vector.tensor_scalar(out=zneg[:], in0=xcol[:], scalar1=-1.0,
                            scalar2=None, op0=ALU.mult)
    nc.vector.tensor_tensor(out=zneg[:], in0=xcol[:], in1=zneg[:], op=ALU.min)
    u = fin.tile([128, MT], F32, tag="u")
    i_exp = nc.scalar.activation(u[:], zneg[:], AF.Exp)
    # keep the Exp after every Sqrt so the act table only switches once
    from concourse.bass import _add_dep_helper
    for si in sqrt_instrs:
        _add_dep_helper(i_exp.ins, si.ins, sync=False, reason="act table order")

    u2 = fin.tile([128, MT], F32, tag="u2")
    nc.vector.tensor_tensor(out=u2[:], in0=u[:], in1=u[:], op=ALU.mult)
    t1 = fin.tile([128, MT], F32, tag="t1")
    nc.vector.tensor_scalar(out=t1[:], in0=u[:], scalar1=60.0, scalar2=60.0,
                            op0=ALU.mult, op1=ALU.add)
    t2 = fin.tile([128, MT], F32, tag="t2")
    nc.vector.tensor_scalar(out=t2[:], in0=u2[:], scalar1=11.0, scalar2=None,
                            op0=ALU.mult)
    nc.vector.tensor_tensor(out=t1[:], in0=t1[:], in1=t2[:], op=ALU.add)
    num = fin.tile([128, MT], F32, tag="num")
    nc.vector.tensor_tensor(out=num[:], in0=t1[:], in1=u[:], op=ALU.mult)
    den = fin.tile([128, MT], F32, tag="den")
    nc.vector.tensor_scalar(out=den[:], in0=u[:], scalar1=90.0, scalar2=60.0,
                            op0=ALU.mult, op1=ALU.add)
    nc.vector.tensor_scalar(out=t2[:], in0=u[:], scalar1=3.0, scalar2=36.0,
                            op0=ALU.mult, op1=ALU.add)
    nc.vector.tensor_tensor(out=t2[:], in0=t2[:], in1=u2[:], op=ALU.mult)
    nc.vector.tensor_tensor(out=den[:], in0=den[:], in1=t2[:], op=ALU.add)
    rden = fin.tile([128, MT], F32, tag="rden")
    nc.vector.reciprocal(rden[:], den[:])
    lg = fin.tile([128, MT], F32, tag="lg")
    nc.vector.tensor_tensor(out=lg[:], in0=num[:], in1=rden[:], op=ALU.mult)
    relu = fin.tile([128, MT], F32, tag="relu")
    nc.vector.tensor_scalar_max(out=relu[:], in0=xcol[:], scalar1=0.0)
    nc.vector.tensor_tensor(out=lg[:], in0=lg[:], in1=relu[:], op=ALU.add)
    lsum = fin.tile([128, 1], F32, tag="lsum")
    nc.vector.tensor_reduce(out=lsum[:], in_=lg[:],
                            axis=mybir.AxisListType.X, op=ALU.add)
    ps = dpool.tile([128, 1024], F32, tag="dif", name="psfinal")
    nc.tensor.matmul(ps[0:1, 0:1], lsum[:], ones_col[:], start=True, stop=True)
    res = fin.tile([1, 1], F32, tag="res")
    nc.scalar.activation(res[:], ps[0:1, 0:1], AF.Copy)
    nc.sync.dma_start(out_d, res[:])


def _get_nc():
    if "nc" not in _CACHE:
        _CACHE["nc"] = _build()
    return _CACHE["nc"]


def _feedback_quant(x):
    """fp8-quantize rows of x with error feedback along the last axis so
    each row's fp8 sum tracks the fp32 row sum."""
    out = np.empty(x.shape, dtype=NPF8)
    carry = np.zeros(x.shape[0], dtype=np.float32)
    for d in range(x.shape[1]):
        v = x[:, d] + carry
        q = v.astype(NPF8)
        out[:, d] = q
        carry = v - q.astype(np.float32)
    return out


def _host_pack(A, P, N):
    Ys = [A, P, N]
    A8 = A.astype(NPF8)

    eye = (np.eye(128, dtype=np.float32) * -224.0).astype(NPF8)
    eyepack = np.ascontiguousarray(np.stack([eye, eye], axis=1))
    sgn = np.where(np.arange(128) % 2 == 0, -224.0, 224.0).astype(np.float32)
    eya = (np.diag(sgn)).astype(NPF8)
    eyealt = np.ascontiguousarray(np.stack([eya, eya], axis=1))
    eyeid = np.eye(128, dtype=np.float16)
    # row d: indicator of its pair column at 512 + d//2; ktile1 = zeros
    ib = np.zeros((128, 2, 1024), dtype=np.float32)
    ib[np.arange(128), 0, 512 + np.arange(128) // 2] = 1.0
    ibufpack = ib.astype(NPF8)

    in_maps = []
    for c in range(NCORES):
        r = RB * c
        idx = np.r_[r:B, 0:r]
        m = {"eyepack": eyepack, "eyealt": eyealt, "eyeid": eyeid,
             "ibufpack": ibufpack}
        for y in range(3):
            Yr = Ys[y][idx]
            ysum = ((Yr[0::2] + Yr[1::2]) * 0.5).astype(NPF8)
            ydif = ((Yr[0::2] - Yr[1::2]) * 0.5).astype(NPF8)
            ysq = Yr * Yr
            qsum = _feedback_quant((ysq[0::2] + ysq[1::2]) * 0.5 - 1.0)
            qdif = _feedback_quant((ysq[0::2] - ysq[1::2]) * 0.5)
            k0 = np.concatenate([ysum.T, ydif.T], axis=1)   # [128, 4096]
            k1 = np.concatenate([qsum.T, qdif.T], axis=1)
            m[f"ypk{y}"] = np.ascontiguousarray(
                np.stack([k0, k1], axis=1)).astype(NPF8)
        ownT = A8[idx][:RB].T          # [128, 512] fp8
        lhspack = np.empty((128, MT, 2, 128), dtype=NPF8)
        for mm in range(MT):
            lhspack[:, mm, 0, :] = ownT[:, 128 * mm:128 * (mm + 1)]
        lhspack[:, :, 1, :] = np.float32(-0.5)
        m["lhspack"] = lhspack
        m["aslice"] = np.ascontiguousarray(A[idx][:RB].T)
        m["pslice"] = np.ascontiguousarray(P[idx][:RB].T)
        in_maps.append(m)
    return in_maps


def kernel(rep_anchor, rep_pos, rep_neg):
    A = np.ascontiguousarray(rep_anchor, dtype=np.float32)
    P = np.ascontiguousarray(rep_pos, dtype=np.float32)
    N = np.ascontiguousarray(rep_neg, dtype=np.float32)

    nc = _get_nc()
    in_maps = _host_pack(A, P, N)
    res = bass_utils.run_bass_kernel_spmd(nc, in_maps,
                                          core_ids=list(range(NCORES)))
    total = np.float64(0.0)
    for c in range(NCORES):
        total += np.float64(res.results[c]["out"][0, 0])
    return np.float32(total / B)
